# revision 4
# baseline (speedup 1.0000x reference)
"""CollisionLoss Trainium2 kernel.

Full inputs -> shard box axis N across 8 NeuronCores -> Bass/Tile kernel
per core -> host gather (sum of per-partition partial sums).

Device layout per core:
  - 12500 boxes per (core, t); T=6 timesteps.
  - SBUF tiles are [126, 598] f32: partition p = t*21 + j  (t in 0..5,
    j in 0..20), free dim f in 0..597; box index within t = j*598 + f.
    21*598 = 12558 >= 12500; the pad slots hold a far-away unit box that
    yields exactly zero penalty (same replacement applied to gt_mask=0).
  - Per-t constants (ego-vehicle circle features) are per-partition [126,1]
    columns, used via activation bias/scale APs and scalar_tensor_tensor.

Math (matches the reference, including its buggy 'width' metric):
  For each box: width  = min_i |dx_i + dy_i| over edges (parallelogram =>
  only edges e0, e1 needed), length^2 Q = max(|e0|^2, |e1|^2), long edge U
  selected by predicated copy.  The 5 circle centers are center + alpha*V,
  V = U * (0.5 - 0.5*width*rsqrt(Q)), alpha in {0, +-1, +-1/2}; same for the
  ego box with G = half*dir (host precomputed), beta in {0, +-1, +-1/2}.
  dist^2(alpha,beta) = D + alpha^2 h^2 + 2 alpha P + beta^2 g^2
                       - 2 beta (R + alpha S)
  with D=|Delta|^2, P=Delta.V, R=Delta.G, S=V.G, h^2=|V|^2, g^2=|G|^2.
  min over beta for fixed alpha:  + min(0, g^2-2|F|, g^2/4-|F|), F=R+alpha*S
    = - max(0, 2|F|-g^2, |F|-g^2/4)   (computed as max of two Relus)
  min over the 5 alphas, + D, clamp, sqrt via exp(0.5*ln(x+eps)),
  pen = relu(0.5*width + 0.5*sdc_w - min_dis), row-summed via accum_out.
"""

import numpy as np

import concourse.bass as bass
import concourse.tile as tile
from concourse import mybir
from concourse.bass_utils import run_bass_kernel_spmd

T = 6
N = 100000
NCORES = 8
NSH = N // NCORES            # boxes per core per t = 12500
PPT = 21                     # partition chunks per t
PT = T * PPT                 # 126 partitions used
FD = 598                     # free dim;  PPT*FD = 12558 >= NSH
NPAD = PPT * FD              # padded boxes per (core, t)
W_EGO = 1.85 + 0.5
L_EGO = 4.084 + 0.5
WEIGHT = 1.0
PADC = 20000.0               # far-away pad box center

OP = mybir.AluOpType
AF = mybir.ActivationFunctionType
F32 = mybir.dt.float32


# ----------------------------------------------------------------------------
# host-side replica of the reference ego(sdc) circle features (T=6 boxes only)
# ----------------------------------------------------------------------------

def _host_make_corners(x, y, w, l, theta):
    hw, hl = w / 2, l / 2
    lx = np.stack([hw, hw, -hw, -hw], axis=-1)
    ly = np.stack([-hl, hl, hl, -hl], axis=-1)
    c, s = np.cos(theta)[..., None], np.sin(theta)[..., None]
    cx = c * lx + s * ly + x[..., None]
    cy = -s * lx + c * ly + y[..., None]
    return np.stack([cx, cy], axis=-1)            # [..., 4, 2]


def _host_circle_feats(corners):
    d_next = corners - np.roll(corners, -1, axis=-2)
    width = np.min(np.abs(np.sum(d_next, axis=-1)), axis=-1)
    e = corners - np.roll(corners, 1, axis=-2)
    elen = np.sqrt(np.sum(e * e, axis=-1))
    length = np.max(elen, axis=-1)
    idx = np.argmax(elen, axis=-1)
    ev = np.take_along_axis(e, np.repeat(idx[..., None, None], 2, axis=-1), axis=-2)[..., 0, :]
    slope = np.arctan(ev[..., 1] / ev[..., 0])
    center = np.mean(corners, axis=-2)
    half = length / 2 - width / 2
    offs = np.stack([np.zeros_like(half), half, -half, half / 2, -half / 2], axis=-1)
    dirv = np.stack([np.cos(slope), np.sin(slope)], axis=-1)
    centers = center[..., None, :] + offs[..., None] * dirv[..., None, :]
    return centers, width                          # [...,5,2], [...]


# ----------------------------------------------------------------------------
# the Bass kernel (built once, cached)
# ----------------------------------------------------------------------------

def _split_waits(nc, max_waits=1):
    """This walrus build only encodes one sync-wait per instruction; hoist
    extra waits onto preceding no-ops on the same engine."""
    for fn in nc.m.functions:
        for bb in fn.blocks:
            new_instrs = []
            for ins in bb.instructions:
                si = ins.sync_info
                if si is not None and si.on_wait and len(si.on_wait) > max_waits:
                    waits = list(si.on_wait)
                    extra, keep = waits[:-max_waits], waits[-max_waits:]
                    for ci in range(0, len(extra), max_waits):
                        new_instrs.append(mybir.InstNoOp(
                            name=f"{ins.name}-ws{ci}", engine=ins.engine,
                            bass_nofuse=True,
                            sync_info=mybir.SyncInfo(
                                on_wait=extra[ci:ci + max_waits], on_update=[])))
                    si.on_wait = keep
                new_instrs.append(ins)
            bb.instructions[:] = new_instrs


def build_nc():
    nc = bass.Bass()
    data = nc.dram_tensor("data", [8, PT, FD], F32, kind="ExternalInput")
    consts = nc.dram_tensor("consts", [PT, 10], F32, kind="ExternalInput")
    out = nc.dram_tensor("acc", [PT, 1], F32, kind="ExternalOutput")

    with tile.TileContext(nc) as tc:
        with tc.tile_pool(name="p", bufs=1) as pool:
            def tl(name, fd=FD, dt=F32):
                return pool.tile([PT, fd], dt, tag=name, name=name)

            # ---- loads --------------------------------------------------
            IN = tl("IN", fd=8 * FD)
            inv = IN[:].rearrange("p (k f) -> p k f", k=8)
            dv = data[:].rearrange("k p f -> p k f")
            # two DMAs so compute on comps 0..3 can start early
            nc.sync.dma_start(inv[:, 0:4, :], dv[:, 0:4, :])
            nc.sync.dma_start(inv[:, 4:8, :], dv[:, 4:8, :])
            C = pool.tile([PT, 10], F32, tag="C", name="C")
            nc.sync.dma_start(C[:], consts[:])

            def comp(k):
                return IN[:, k * FD:(k + 1) * FD]
            X0, Y0, X1, Y1 = comp(0), comp(1), comp(2), comp(3)
            X2, Y2, X3, Y3 = comp(4), comp(5), comp(6), comp(7)
            negscx, negscy = C[:, 0:1], C[:, 1:2]
            Gx, Gy = C[:, 2:3], C[:, 3:4]
            negqg2, negg2, chalf = C[:, 4:5], C[:, 5:6], C[:, 6:7]
            half_c, eps_c = C[:, 7:8], C[:, 8:9]

            V, S, G = nc.vector, nc.scalar, nc.gpsimd

            # ---- width chain (gpsimd + act) -----------------------------
            u0 = tl("u0"); G.tensor_tensor(u0[:], X0[:], Y0[:], OP.add)
            u1 = tl("u1"); G.tensor_tensor(u1[:], X1[:], Y1[:], OP.add)
            u3 = tl("u3"); G.tensor_tensor(u3[:], X3[:], Y3[:], OP.add)
            # w0 -> u3 (in place), w1 -> u1
            G.tensor_tensor(u3[:], u0[:], u3[:], OP.subtract)
            G.tensor_tensor(u1[:], u1[:], u0[:], OP.subtract)
            aw0 = tl("aw0"); S.activation(aw0[:], u3[:], AF.Abs)
            aw1 = tl("aw1"); S.activation(aw1[:], u1[:], AF.Abs)
            width = aw0  # min in place
            V.tensor_tensor(width[:], aw0[:], aw1[:], OP.min)

            # ---- center chain (gpsimd + act) ----------------------------
            sxa = tl("sxa"); G.tensor_tensor(sxa[:], X0[:], X1[:], OP.add)
            sxb = tl("sxb"); G.tensor_tensor(sxb[:], X2[:], X3[:], OP.add)
            G.tensor_tensor(sxa[:], sxa[:], sxb[:], OP.add)      # sx -> sxa
            sya = tl("sya"); G.tensor_tensor(sya[:], Y0[:], Y1[:], OP.add)
            syb = tl("syb"); G.tensor_tensor(syb[:], Y2[:], Y3[:], OP.add)
            G.tensor_tensor(sya[:], sya[:], syb[:], OP.add)      # sy -> sya
            dx = sxb; dy = syb  # reuse the dead halves as outputs
            S.activation(dx[:], sxa[:], AF.Identity, bias=negscx, scale=0.25)
            S.activation(dy[:], sya[:], AF.Identity, bias=negscy, scale=0.25)

            # ---- edges / length / long-edge select (vector + act) -------
            ex0 = tl("ex0"); V.tensor_tensor(ex0[:], X0[:], X3[:], OP.subtract)
            ey0 = tl("ey0"); V.tensor_tensor(ey0[:], Y0[:], Y3[:], OP.subtract)
            ex1 = tl("ex1"); V.tensor_tensor(ex1[:], X1[:], X0[:], OP.subtract)
            ey1 = tl("ey1"); V.tensor_tensor(ey1[:], Y1[:], Y0[:], OP.subtract)
            qx0 = tl("qx0"); S.activation(qx0[:], ex0[:], AF.Square)
            qy0 = tl("qy0"); S.activation(qy0[:], ey0[:], AF.Square)
            qx1 = tl("qx1"); S.activation(qx1[:], ex1[:], AF.Square)
            qy1 = tl("qy1"); S.activation(qy1[:], ey1[:], AF.Square)
            V.tensor_tensor(qx0[:], qx0[:], qy0[:], OP.add)      # q0 -> qx0
            V.tensor_tensor(qx1[:], qx1[:], qy1[:], OP.add)      # q1 -> qx1
            q0, q1 = qx0, qx1
            Q = qy0  # reuse
            V.tensor_tensor(Q[:], q0[:], q1[:], OP.max)
            cB = pool.tile([PT, FD], mybir.dt.uint8, tag="cB", name="cB")
            V.tensor_tensor(cB[:], q1[:], q0[:], OP.is_ge)
            Ux, Uy = ex0, ey0  # predicated overwrite selects e1 where q1>=q0
            V.copy_predicated(Ux[:], cB[:], ex1[:])
            V.copy_predicated(Uy[:], cB[:], ey1[:])

            # ---- scale, V, h^2 -----------------------------------------
            lq = qy1  # reuse
            S.activation(lq[:], Q[:], AF.Ln)
            rL = lq
            S.activation(rL[:], lq[:], AF.Exp, bias=0.0, scale=-0.5)
            wr = rL
            V.tensor_tensor(wr[:], width[:], rL[:], OP.mult)
            sc = wr
            S.activation(sc[:], wr[:], AF.Identity, bias=half_c, scale=-0.5)
            Vx = ex1; Vy = ey1  # reuse dead edge tiles
            V.tensor_tensor(Vx[:], Ux[:], sc[:], OP.mult)
            V.tensor_tensor(Vy[:], Uy[:], sc[:], OP.mult)
            scq = tl("scq"); S.activation(scq[:], sc[:], AF.Square)
            h2 = scq
            V.tensor_tensor(h2[:], scq[:], Q[:], OP.mult)

            # ---- D, P, R, S --------------------------------------------
            dxx = Ux  # Ux dead after Vx
            S.activation(dxx[:], dx[:], AF.Square)
            dyy = Uy
            S.activation(dyy[:], dy[:], AF.Square)
            D = dxx
            V.tensor_tensor(D[:], dxx[:], dyy[:], OP.add)
            p1 = tl("p1"); V.tensor_tensor(p1[:], dx[:], Vx[:], OP.mult)
            p2 = tl("p2"); V.tensor_tensor(p2[:], dy[:], Vy[:], OP.mult)
            P = p1
            V.tensor_tensor(P[:], p1[:], p2[:], OP.add)
            r2 = tl("r2"); S.activation(r2[:], dy[:], AF.Identity, bias=0.0, scale=Gy)
            R = r2
            V.scalar_tensor_tensor(R[:], dx[:], Gx, r2[:], OP.mult, OP.add)
            s2 = tl("s2"); S.activation(s2[:], Vy[:], AF.Identity, bias=0.0, scale=Gy)
            S_ = s2
            V.scalar_tensor_tensor(S_[:], Vx[:], Gx, s2[:], OP.mult, OP.add)

            # ---- E' and F per alpha ------------------------------------
            E1p = tl("E1p"); V.scalar_tensor_tensor(E1p[:], P[:], 2.0, h2[:], OP.mult, OP.add)
            E1m = tl("E1m"); V.scalar_tensor_tensor(E1m[:], P[:], -2.0, h2[:], OP.mult, OP.add)
            h2q = tl("h2q"); V.tensor_scalar(h2q[:], h2[:], 0.25, 0.0, OP.mult, OP.add)
            Ehp = tl("Ehp"); V.tensor_tensor(Ehp[:], h2q[:], P[:], OP.add)
            Ehm = h2q
            V.tensor_tensor(Ehm[:], h2q[:], P[:], OP.subtract)
            F1p = tl("F1p"); V.tensor_tensor(F1p[:], R[:], S_[:], OP.add)
            F1m = tl("F1m"); V.tensor_tensor(F1m[:], R[:], S_[:], OP.subtract)
            F2p = tl("F2p"); V.scalar_tensor_tensor(F2p[:], S_[:], 0.5, R[:], OP.mult, OP.add)
            F2m = tl("F2m"); V.scalar_tensor_tensor(F2m[:], S_[:], -0.5, R[:], OP.mult, OP.add)

            # ---- per-alpha beta-collapse:  tot_j = E'_j - max(0, relu1, relu2)
            js = [("0", R, None), ("1p", F1p, E1p), ("1m", F1m, E1m),
                  ("hp", F2p, Ehp), ("hm", F2m, Ehm)]
            tots = {}
            for tag, Fj, Ej in js:
                ph = tl("ph" + tag); S.activation(ph[:], Fj[:], AF.Abs)
                n1 = tl("n1" + tag); S.activation(n1[:], ph[:], AF.Relu, bias=negqg2, scale=1.0)
                n2 = ph  # reuse
                S.activation(n2[:], ph[:], AF.Relu, bias=negg2, scale=2.0)
                Mz = n1  # max in place
                V.tensor_tensor(Mz[:], n1[:], n2[:], OP.max)
                tot = Mz  # subtract writes over Mz (reversed operand order safe)
                if Ej is None:
                    # alpha = 0: E' = 0, fold the final +D here
                    V.tensor_tensor(tot[:], D[:], Mz[:], OP.subtract)
                else:
                    V.tensor_tensor(tot[:], Ej[:], Mz[:], OP.subtract)
                tots[tag] = tot

            m1 = tots["1p"]
            V.tensor_tensor(m1[:], tots["1p"][:], tots["1m"][:], OP.min)
            m2 = tots["hp"]
            V.tensor_tensor(m2[:], tots["hp"][:], tots["hm"][:], OP.min)
            m3 = m1
            V.tensor_tensor(m3[:], m1[:], m2[:], OP.min)
            md2 = m3
            V.tensor_tensor(md2[:], D[:], m3[:], OP.add)
            V.tensor_tensor(md2[:], md2[:], tots["0"][:], OP.min)

            # ---- sqrt via exp/ln, penalty, row-sum ---------------------
            S.activation(md2[:], md2[:], AF.Relu)
            S.activation(md2[:], md2[:], AF.Ln, bias=eps_c, scale=1.0)
            md = md2
            S.activation(md[:], md2[:], AF.Exp, bias=0.0, scale=0.5)
            wm = md
            V.scalar_tensor_tensor(wm[:], md[:], -2.0, width[:], OP.mult, OP.add)
            pen = wm
            acc = pool.tile([PT, 1], F32, tag="accT", name="accT")
            S.activation(pen[:], wm[:], AF.Relu, bias=chalf, scale=0.5,
                         accum_out=acc[:, 0:1])
            nc.sync.dma_start(out[:], acc[:])

    _split_waits(nc)
    return nc


_NC_CACHE = None


def _get_nc():
    global _NC_CACHE
    if _NC_CACHE is None:
        _NC_CACHE = build_nc()
    return _NC_CACHE


# ----------------------------------------------------------------------------
# host wrapper
# ----------------------------------------------------------------------------

def _prep_inputs(sdc_traj_all, sdc_planning_gt, gt_corners, gt_mask):
    # ego circle features (T=6) — replicate reference math on host
    x = np.asarray(sdc_traj_all, dtype=np.float64)[0, :, 0]
    y = np.asarray(sdc_traj_all, dtype=np.float64)[0, :, 1]
    theta = np.asarray(sdc_planning_gt, dtype=np.float64)[0, :, 2]
    w = np.full_like(x, W_EGO)
    l = np.full_like(x, L_EGO)
    sdc_corners = _host_make_corners(x, y, w, l, theta)        # [T,4,2]
    sdc_centers, sdc_w = _host_circle_feats(sdc_corners)       # [T,5,2],[T]
    scx = sdc_centers[:, 0, 0]
    scy = sdc_centers[:, 0, 1]
    Gx = sdc_centers[:, 1, 0] - scx
    Gy = sdc_centers[:, 1, 1] - scy
    g2 = Gx * Gx + Gy * Gy

    cols = np.zeros((T, 10), dtype=np.float64)
    cols[:, 0] = -scx
    cols[:, 1] = -scy
    cols[:, 2] = Gx
    cols[:, 3] = Gy
    cols[:, 4] = -0.25 * g2
    cols[:, 5] = -g2
    cols[:, 6] = 0.5 * sdc_w
    cols[:, 7] = 0.5
    cols[:, 8] = 1e-12
    consts = np.repeat(cols[:, None, :], PPT, axis=1).reshape(PT, 10).astype(np.float32)

    # pad/masked replacement box: unit square at (PADC, PADC)
    padvals = np.array([PADC + .5, PADC - .5, PADC + .5, PADC + .5,
                        PADC - .5, PADC + .5, PADC - .5, PADC - .5],
                       dtype=np.float32)             # X0 Y0 X1 Y1 X2 Y2 X3 Y3

    gt = np.asarray(gt_corners, dtype=np.float32)    # [T,N,4,2]
    gm = np.asarray(gt_mask).astype(bool)            # [T,N]

    in_maps = []
    for c in range(NCORES):
        sl = slice(c * NSH, (c + 1) * NSH)
        gtc = gt[:, sl]                              # [T,NSH,4,2]
        gmc = gm[:, sl]                              # [T,NSH]
        comps = gtc.reshape(T, NSH, 8).transpose(2, 0, 1)   # [8,T,NSH]
        data = np.empty((8, T, NPAD), dtype=np.float32)
        data[:, :, NSH:] = padvals[:, None, None]
        keep = gmc[None, :, :]
        data[:, :, :NSH] = np.where(keep, comps, padvals[:, None, None])
        data = np.ascontiguousarray(data.reshape(8, PT, FD))
        in_maps.append({"data": data, "consts": consts})
    return in_maps


def kernel(sdc_traj_all, sdc_planning_gt, sdc_planning_gt_mask, gt_corners,
           gt_mask, _trace=False, _trace_kwargs=None):
    nc = _get_nc()
    in_maps = _prep_inputs(sdc_traj_all, sdc_planning_gt, gt_corners, gt_mask)
    kw = {}
    if _trace:
        kw = dict(trace=True, **(_trace_kwargs or {}))
    res = run_bass_kernel_spmd(nc, in_maps, list(range(NCORES)), **kw)
    total = np.float32(0.0)
    for r in res.results:
        total = np.float32(total + np.float32(r["acc"].sum(dtype=np.float32)))
    out = np.array([total * np.float32(WEIGHT)], dtype=np.float32)
    if _trace:
        return out, res
    return out
